# revision 1
# baseline (speedup 1.0000x reference)
"""GCMC GraphConv kernel for 8 Trainium2 NeuronCores.

Computation:  out = ci * segment_sum((input_feat @ weight * cj)[src], dst)

Strategy (dst-sharded, no collectives):
  - Nodes are 1D-partitioned by destination: core c owns dst rows
    [c*N/8, (c+1)*N/8).  Each edge is routed (on host) to the core owning
    its destination, so no cross-core reduction is needed.
  - Per core: h = (X @ W) * cj is computed for ALL nodes on the PE engine
    (X^T is replicated) and stored in HBM, split into 4 windows of 25000
    rows so gathers can use int16 indices and pipeline behind the matmul.
  - The per-edge gather h[src] uses the SWDGE dma_gather instruction
    (GPSIMD generates one 256B descriptor per edge).
  - The per-edge scatter-add over dst is done on the PE engine: edges are
    host-sorted by (dst block of 128, src window); for each 128-edge tile a
    one-hot matrix onehot[e, n] = (dst_local[e] == n) is built on the DVE
    (is_equal against an iota row) and matmul-accumulated into a PSUM tile
    for that dst block.  PSUM is flushed into an SBUF accumulator.
    This avoids any read-modify-write races that a DMA scatter-add with
    duplicate indices would have.
  - Finally acc is scaled by ci and written out; the host concatenates the
    8 core outputs.

The per-(window, block) tile count must be identical on all cores (single
SPMD program), so each group is padded to the max count over cores; pad
edges gather row 0 of the window and carry dst = -1 (never matches the
one-hot compare, so they contribute exactly zero).
"""

import dataclasses
import math

import numpy as np
import ml_dtypes

import concourse.bacc as bacc
import concourse.bass as bass
import concourse.mybir as mybir
import concourse.tile as tile
from concourse.bass_utils import run_bass_kernel_spmd

BF16 = ml_dtypes.bfloat16
P = 128
NCORES = 8
D_IN = 128


@dataclasses.dataclass(frozen=True)
class Cfg:
    N: int = 100000
    D_OUT: int = 64          # 64 * 4B = 256B rows (dma_gather granularity)
    NWIN: int = 4            # src windows; N/NWIN must be < 32768 (int16 idx)
    SUPER: int = 2048        # phase-1 node supertile (cols of X^T per DMA)
    MAX_CHUNK_TILES: int = 8   # gather chunk (1024 descs = SWDGE ring capacity)
    NQUEUES: int = 2         # SWDGE queues; alternate so desc-gen overlaps DMA
    SCRATCH: int = 32768     # dynamic DMA descriptor carveout (bytes/partition)
    H_BF16: bool = True      # compute h = X@W in bf16 (PE fp32 matmul is 4x slower)
    INDIRECT: bool = False   # indirect_dma_start gather: correct in CoreSim but
    #   returns garbage on this HW stack (no ucode support) — keep dma_gather

    @property
    def n_loc(self):
        return self.N // NCORES

    @property
    def nblk(self):
        return math.ceil(self.n_loc / P)

    @property
    def win(self):
        return self.N // self.NWIN

    @property
    def tpw(self):
        return math.ceil(self.win / P)  # node tiles per window


CFG = Cfg()


# ---------------------------------------------------------------- host prep

def shard_edges(cfg: Cfg, src, dst):
    """Route and sort edges; build per-core padded index/dst arrays.

    Destination nodes are partitioned into 128-row blocks; blocks are
    assigned to (core, slot) pairs by sorted edge count so that each slot's
    8 blocks (one per core) have similar counts — the SPMD program pads
    every slot to the max over its 8 cores, so balanced slots minimize
    gather padding (~7% vs ~25% for contiguous assignment).

    Returns (G, per_core, block_of):
      G[w][s]       tiles of (window w, slot s) — identical across cores
      per_core[c]   dict with idx{w} / dstb{w} device arrays
      block_of[c,s] global dst block handled by core c, slot s
    """
    nblk, win, nw_ = cfg.nblk, cfg.win, cfg.NWIN
    src = np.asarray(src, dtype=np.int64)
    dst = np.asarray(dst, dtype=np.int64)
    gb = dst >> 7                            # global dst block
    dstb = (dst & 127).astype(np.float32)    # dst within block
    wine = src // win
    src_loc = (src - wine * win).astype(np.int16)

    nblk_g = NCORES * nblk                   # padded global block count
    bc = np.bincount(gb, minlength=nblk_g)   # edges per global block
    order = np.argsort(-bc, kind="stable")   # blocks by descending count
    # slot s gets ranked blocks [8s, 8s+8); core = position in the group
    block_of = np.empty((NCORES, nblk), dtype=np.int64)
    block_core = np.empty(nblk_g, dtype=np.int64)
    block_slot = np.empty(nblk_g, dtype=np.int64)
    for s in range(nblk):
        grp = order[s * NCORES:(s + 1) * NCORES]
        block_of[:, s] = grp
        block_core[grp] = np.arange(NCORES)
        block_slot[grp] = s

    core = block_core[gb]
    slot = block_slot[gb]

    gid = (core * nw_ + wine) * nblk + slot
    counts = np.bincount(gid, minlength=NCORES * nw_ * nblk)
    counts = counts.reshape(NCORES, nw_, nblk)
    G = -(-counts.max(axis=0) // P)          # ceil tiles per (w, slot)
    G[0] = np.maximum(G[0], 1)               # w=0 flush initializes acc
    tiles_w = G.sum(axis=1)                  # [NWIN]

    off_ws = np.zeros((nw_, nblk), dtype=np.int64)
    off_ws[:, 1:] = np.cumsum(G[:, :-1], axis=1) * P

    per_core = []
    for c in range(NCORES):
        m = core == c
        sl, db, we, bl = src_loc[m], dstb[m], wine[m], slot[m]
        key = we * nblk + bl
        o = np.argsort(key, kind="stable")
        ks = key[o]
        gcnt = np.bincount(ks, minlength=nw_ * nblk)
        gstart = np.concatenate([[0], np.cumsum(gcnt)[:-1]])
        within = np.arange(ks.size) - gstart[ks]
        wsel, ssel = ks // nblk, ks % nblk
        pos = off_ws[wsel, ssel] + within
        maps = {}
        for w in range(nw_):
            nw_edges = int(tiles_w[w]) * P
            ia = np.zeros(nw_edges, dtype=np.int16)        # pad -> row 0
            da = np.full(nw_edges, -1.0, dtype=np.float32)  # pad -> no match
            sel = wsel == w
            ia[pos[sel]] = sl[o][sel]
            da[pos[sel]] = db[o][sel]
            if cfg.INDIRECT:
                # msgs-layout int32 offsets: edge j -> [j%128, j//128]
                maps[f"idx{w}"] = np.ascontiguousarray(
                    ia.astype(np.int32).reshape(-1, P).T)
            else:
                maps[f"idx{w}"] = np.ascontiguousarray(
                    np.tile(ia.reshape(-1, 16).T, (8, 1)))
            maps[f"dstb{w}"] = np.ascontiguousarray(da.reshape(-1, P).T)
        per_core.append(maps)
    return G, per_core, block_of


def host_inputs(cfg: Cfg, input_feat, weight, cj, ci, block_of):
    """Shared (replicated) device inputs + per-core civ (slot layout)."""
    N, dout, nblk = cfg.N, cfg.D_OUT, cfg.nblk
    mm_dt = BF16 if cfg.H_BF16 else np.float32
    xt = np.ascontiguousarray(np.asarray(input_feat, dtype=np.float32).T.astype(mm_dt))
    wgt = np.ascontiguousarray(np.asarray(weight, dtype=np.float32).astype(mm_dt))
    # cj laid out per (window, tile): col w*tpw+tt, partition = row in tile
    cjf = np.asarray(cj, dtype=np.float32).reshape(-1)
    ncols = cfg.NWIN * cfg.tpw
    cjv = np.zeros((P, ncols), dtype=np.float32)
    for w in range(cfg.NWIN):
        wvals = cjf[w * cfg.win:(w + 1) * cfg.win]
        pad = np.zeros(cfg.tpw * P, dtype=np.float32)
        pad[:wvals.size] = wvals
        cjv[:, w * cfg.tpw:(w + 1) * cfg.tpw] = pad.reshape(cfg.tpw, P).T
    iot = np.ascontiguousarray(
        np.broadcast_to(np.arange(P, dtype=np.float32), (P, P)).astype(BF16))
    # ci in (core, slot) layout: civ[c][p, s] = ci[block_of[c,s]*128 + p]
    cip = np.zeros(NCORES * nblk * P, dtype=np.float32)
    cif = np.asarray(ci, dtype=np.float32).reshape(-1)
    cip[:N] = cif
    cip = cip.reshape(NCORES * nblk, P)
    civs = [np.ascontiguousarray(cip[block_of[c]].T) for c in range(NCORES)]
    return {"xt": xt, "wgt": wgt, "cjv": cjv, "iot": iot}, civs


# ---------------------------------------------------------------- device IR

def tile_blocks(cfg: Cfg, G, w):
    """Per-tile (block, k, g) for window w, in edge order."""
    out = []
    for b in range(cfg.nblk):
        g = int(G[w][b])
        for k in range(g):
            out.append((b, k, g))
    return out


def build_nc(cfg: Cfg, G):
    f32, bf16, i16 = mybir.dt.float32, mybir.dt.bfloat16, mybir.dt.int16
    dout, nblk, win, tpw = cfg.D_OUT, cfg.nblk, cfg.win, cfg.tpw
    tiles_w = [int(sum(G[w])) for w in range(cfg.NWIN)]

    nc = bacc.Bacc("TRN2", target_bir_lowering=False, debug=False,
                   num_swdge_queues=cfg.NQUEUES,
                   dynamic_dma_scratch_size=cfg.SCRATCH)
    mm_dt = bf16 if cfg.H_BF16 else f32
    xt = nc.dram_tensor("xt", [D_IN, cfg.N], mm_dt, kind="ExternalInput")
    wgt = nc.dram_tensor("wgt", [D_IN, dout], mm_dt, kind="ExternalInput")
    cjv = nc.dram_tensor("cjv", [P, cfg.NWIN * tpw], f32, kind="ExternalInput")
    civ = nc.dram_tensor("civ", [P, nblk], f32, kind="ExternalInput")
    iot = nc.dram_tensor("iot", [P, P], bf16, kind="ExternalInput")
    i32 = mybir.dt.int32
    if cfg.INDIRECT:
        idx_t = [nc.dram_tensor(f"idx{w}", [P, tiles_w[w]], i32,
                                kind="ExternalInput") for w in range(cfg.NWIN)]
    else:
        idx_t = [nc.dram_tensor(f"idx{w}", [P, tiles_w[w] * 8], i16,
                                kind="ExternalInput") for w in range(cfg.NWIN)]
    dstb_t = [nc.dram_tensor(f"dstb{w}", [P, tiles_w[w]], f32,
                             kind="ExternalInput") for w in range(cfg.NWIN)]
    hw_t = [nc.dram_tensor(f"h{w}", [tpw * P, dout], f32)
            for w in range(cfg.NWIN)]
    out_t = nc.dram_tensor("out", [nblk * P, dout], f32, kind="ExternalOutput")

    with tile.TileContext(nc) as tc:
        with (
            tc.tile_pool(name="const", bufs=1) as cpool,
            tc.tile_pool(name="xt", bufs=3) as xpool,
            tc.tile_pool(name="hs", bufs=3) as hpool,
            tc.tile_pool(name="idx", bufs=2) as ipool,
            tc.tile_pool(name="msg", bufs=2) as mpool,
            tc.tile_pool(name="oh", bufs=6) as opool,
            tc.tile_pool(name="ps", bufs=4, space="PSUM") as pspool,
            tc.tile_pool(name="acc", bufs=1) as apool,
        ):
            wgt_sb = cpool.tile([P, dout], mm_dt, tag="wgt")
            nc.sync.dma_start(out=wgt_sb[:], in_=wgt[:])
            iota_sb = cpool.tile([P, P], bf16, tag="iot")
            nc.sync.dma_start(out=iota_sb[:], in_=iot[:])
            cj_sb = cpool.tile([P, cfg.NWIN * tpw], f32, tag="cj")
            nc.sync.dma_start(out=cj_sb[:], in_=cjv[:])
            ci_sb = cpool.tile([P, nblk], f32, tag="ci")
            nc.sync.dma_start(out=ci_sb[:], in_=civ[:])
            acc = apool.tile([P, nblk * dout], f32, tag="acc")

            def emit_super(w, s):
                """Phase 1: one supertile of h_w = (X @ W) * cj."""
                ncols = min(cfg.SUPER, win - s)
                nsub = math.ceil(ncols / P)
                xt_sb = xpool.tile([P, cfg.SUPER], mm_dt, tag="xt")
                nc.sync.dma_start(out=xt_sb[:, :ncols],
                                  in_=xt[:, w * win + s:w * win + s + ncols])
                hs_sb = hpool.tile([P, (cfg.SUPER // P) * dout], f32, tag="hs")
                for q in range(nsub):
                    rows = min(P, ncols - q * P)
                    ps = pspool.tile([P, dout], f32, tag="ps1")
                    nc.tensor.matmul(
                        out=ps[:rows],
                        lhsT=xt_sb[:, q * P:q * P + rows],
                        rhs=wgt_sb[:],
                        start=True, stop=True)
                    tt = (s + q * P) // P  # tile index within window
                    nc.scalar.mul(
                        hs_sb[:rows, q * dout:(q + 1) * dout],
                        ps[:rows],
                        cj_sb[:rows, w * tpw + tt:w * tpw + tt + 1])
                nfull = ncols // P  # full 128-row subtiles
                if nfull:
                    nc.sync.dma_start(
                        out=hw_t[w][s:s + nfull * P, :].rearrange(
                            "(q p) f -> p q f", p=P),
                        in_=hs_sb[:, :nfull * dout].rearrange(
                            "p (q f) -> p q f", f=dout))
                if nsub > nfull:  # ragged tail: only the valid rows
                    rows = ncols - nfull * P
                    nc.sync.dma_start(
                        out=hw_t[w][s + nfull * P:s + ncols, :],
                        in_=hs_sb[:rows, nfull * dout:nsub * dout])

            st = {"ps": None}

            def emit_chunk(w, t0, tb, idx_sb, dst_sb, qn):
                """Phase 2: gather one chunk of edges and matmul-scatter it."""
                t1 = min(t0 + cfg.MAX_CHUNK_TILES, len(tb))
                nt = t1 - t0
                ne = nt * P
                msg = mpool.tile([P, nt * dout], f32, tag="msg")
                if cfg.INDIRECT:
                    nc.gpsimd.indirect_dma_start(
                        out=msg[:].rearrange("p (t f) -> p t f", f=dout),
                        out_offset=None,
                        in_=hw_t[w][:],
                        in_offset=bass.IndirectOffsetOnAxis(
                            ap=idx_sb[:, t0:t1], axis=0))
                else:
                    nc.gpsimd.dma_gather(
                        msg[:].rearrange("p (t f) -> p t f", f=dout),
                        hw_t[w][:],
                        idx_sb[:, t0 * 8:t1 * 8],
                        ne, ne, dout,
                        queue_num=qn)
                msg16 = mpool.tile([P, nt * dout], bf16, tag="msg16")
                nc.scalar.copy(msg16[:], msg[:])
                for t in range(t0, t1):
                    b, k, g = tb[t]
                    if k == 0:
                        st["ps"] = pspool.tile([P, dout], f32, tag="ps2",
                                               name="ps2")
                    ps = st["ps"]
                    oh = opool.tile([P, P], bf16, tag="oh")
                    nc.vector.tensor_tensor(
                        out=oh[:],
                        in0=dst_sb[:, t:t + 1].to_broadcast([P, P]),
                        in1=iota_sb[:],
                        op=mybir.AluOpType.is_equal)
                    nc.tensor.matmul(
                        out=ps[:],
                        lhsT=oh[:],
                        rhs=msg16[:, (t - t0) * dout:(t - t0 + 1) * dout],
                        start=(k == 0), stop=(k == g - 1))
                    if k == g - 1:
                        if w == 0:
                            nc.vector.tensor_copy(
                                out=acc[:, b * dout:(b + 1) * dout], in_=ps[:])
                        else:
                            nc.vector.tensor_add(
                                out=acc[:, b * dout:(b + 1) * dout],
                                in0=acc[:, b * dout:(b + 1) * dout],
                                in1=ps[:])

            super_starts = list(range(0, win, cfg.SUPER))
            for s in super_starts:
                emit_super(0, s)
            qn = 0
            for w in range(cfg.NWIN):
                if cfg.INDIRECT:
                    idx_sb = ipool.tile([P, tiles_w[w]], i32, tag="idx")
                else:
                    idx_sb = ipool.tile([P, tiles_w[w] * 8], i16, tag="idx")
                nc.sync.dma_start(out=idx_sb[:], in_=idx_t[w][:])
                dst_sb = ipool.tile([P, tiles_w[w]], f32, tag="dstb")
                nc.sync.dma_start(out=dst_sb[:], in_=dstb_t[w][:])

                tb = tile_blocks(cfg, G, w)
                chunk_starts = list(range(0, len(tb), cfg.MAX_CHUNK_TILES))
                # software-pipeline: weave window w+1's phase 1 between
                # window w's gather/scatter chunks so PE/ACT/DMA stay busy
                # while the gather queue drains
                nxt = super_starts if w + 1 < cfg.NWIN else []
                frac, si = 0.0, 0
                ratio = len(nxt) / max(1, len(chunk_starts))
                for t0 in chunk_starts:
                    emit_chunk(w, t0, tb, idx_sb, dst_sb, qn)
                    qn = (qn + 1) % cfg.NQUEUES
                    frac += ratio
                    while frac >= 1.0 and si < len(nxt):
                        emit_super(w + 1, nxt[si])
                        si += 1
                        frac -= 1.0
                while si < len(nxt):
                    emit_super(w + 1, nxt[si])
                    si += 1

            # ---- epilogue: scale by ci, store ----
            for b in range(nblk):
                nc.vector.tensor_mul(
                    out=acc[:, b * dout:(b + 1) * dout],
                    in0=acc[:, b * dout:(b + 1) * dout],
                    in1=ci_sb[:, b:b + 1].to_broadcast([P, dout]))
            nc.sync.dma_start(
                out=out_t[:].rearrange("(b p) f -> p b f", p=P),
                in_=acc[:].rearrange("p (b f) -> p b f", f=dout))
    nc.compile()
    return nc


# ---------------------------------------------------------------- entry

def run(cfg: Cfg, input_feat, weight, cj, ci, src_idx, dst_idx, **run_kwargs):
    G, per_core, block_of = shard_edges(cfg, src_idx, dst_idx)
    shared, civs = host_inputs(cfg, input_feat, weight, cj, ci, block_of)
    nc = build_nc(cfg, G)
    in_maps = []
    for c in range(NCORES):
        m = dict(shared)
        m["civ"] = civs[c]
        m.update(per_core[c])
        in_maps.append(m)
    res = run_bass_kernel_spmd(nc, in_maps, core_ids=list(range(NCORES)),
                               **run_kwargs)
    # un-permute: core c slot s holds global dst block block_of[c, s]
    full = np.zeros((NCORES * cfg.nblk * P, cfg.D_OUT), dtype=np.float32)
    blk_rows = full.reshape(NCORES * cfg.nblk, P, cfg.D_OUT)
    for c in range(NCORES):
        o = res.results[c]["out"].reshape(cfg.nblk, P, cfg.D_OUT)
        blk_rows[block_of[c]] = o
    return full[:cfg.N], res


def kernel(input_feat, weight, cj, ci, src_idx, dst_idx):
    out, _ = run(CFG, input_feat, weight, cj, ci, src_idx, dst_idx)
    return out



# revision 5
# speedup vs baseline: 1.0090x; 1.0090x over previous
"""GCMC GraphConv kernel for 8 Trainium2 NeuronCores.

Computation:  out = ci * segment_sum((input_feat @ weight * cj)[src], dst)

Strategy (dst-sharded, no collectives):
  - Nodes are 1D-partitioned by destination: core c owns the dst blocks
    assigned to it; each edge is routed (on host) to the core owning its
    destination, so no cross-core reduction is needed.
  - Per core: h = (X @ W) * cj is computed for ALL nodes on the PE engine
    (X^T is replicated) and stored in HBM as [*, 128] bf16 rows (256B; cols
    64..127 junk) split into 4 windows of 25000 rows so gathers use int16
    indices and pipeline behind the matmul.
  - The per-edge gather h[src] uses the SWDGE dma_gather instruction
    (GPSIMD generates one 256B descriptor per edge).  The gathered msgs
    are bf16 and feed the scatter matmul directly (no conversion copy).
  - The per-edge scatter-add over dst is done on the PE engine: edges are
    host-sorted by (dst block of 128, src window); for each 128-edge tile a
    one-hot matrix onehot[e, n] = (dst_local[e] == n) is built on the DVE
    (is_equal against an iota row) and matmul-accumulated into a PSUM tile
    for that dst block.  PSUM is flushed into an SBUF accumulator.
  - Finally acc is scaled by ci and written out; the host concatenates the
    8 core outputs.

Padding/cost notes (from the dma_gather ucode): descriptor generation runs
on ONE Q7 core pair (~6.6ns/edge) and is the kernel bottleneck, and the
ucode trims TRAILING negative indices before generating descriptors.  So
gather chunks are aligned to (window, slot) group boundaries and each
core marks its per-chunk trailing padding with idx=-1: that padding costs
zero descriptor time.  Interior padding uses idx=0 (safe row-0 read) and
dst=-1 (zero one-hot => contributes nothing; gathered garbage is harmless).
Slots inside a window are ordered by expected padding (ascending) so the
high-pad slot sits at each chunk's tail where the trim applies.
"""

import dataclasses
import math

import numpy as np
import ml_dtypes

import concourse.bacc as bacc
import concourse.bass as bass
import concourse.mybir as mybir
import concourse.tile as tile
from concourse.bass_utils import run_bass_kernel_spmd

BF16 = ml_dtypes.bfloat16
P = 128
NCORES = 8
D_IN = 128
HROW = 128               # h row width in bf16 (256B, gather granularity)


@dataclasses.dataclass(frozen=True)
class Cfg:
    N: int = 100000
    D_OUT: int = 64
    NWIN: int = 4            # src windows; N/NWIN must be < 32768 (int16 idx)
    SUPER: int = 2048        # phase-1 node supertile (cols of X^T per DMA)
    MAX_CHUNK_TILES: int = 8   # gather chunk cap (1024 descs = SWDGE ring capacity)
    NQUEUES: int = 2         # SWDGE queues; alternate so desc-gen overlaps DMA
    SCRATCH: int = 32768     # dynamic DMA descriptor carveout (bytes/partition)

    @property
    def n_loc(self):
        return self.N // NCORES

    @property
    def nblk(self):
        return math.ceil(self.n_loc / P)

    @property
    def win(self):
        return self.N // self.NWIN

    @property
    def tpw(self):
        return math.ceil(self.win / P)  # node tiles per window


CFG = Cfg()


# ---------------------------------------------------------------- host prep

def shard_edges(cfg: Cfg, src, dst):
    """Route and sort edges; build chunk structure + per-core padded arrays.

    Returns (G, chunk_slots, per_core, block_of):
      G[w][s]          tiles of (window w, slot s) — identical across cores
      chunk_slots[w]   list of chunks; each chunk = list of slot ids
      per_core[c]      dict with idx{w} / dstb{w} device arrays
      block_of[c,s]    global dst block handled by core c, slot s
    """
    nblk, win, nw_ = cfg.nblk, cfg.win, cfg.NWIN
    src = np.asarray(src, dtype=np.int64)
    dst = np.asarray(dst, dtype=np.int64)
    gb = dst >> 7                            # global dst block
    dstb = (dst & 127).astype(np.float32)    # dst within block
    wine = src // win
    src_loc = (src - wine * win).astype(np.int16)

    nblk_g = NCORES * nblk                   # padded global block count
    bc = np.bincount(gb, minlength=nblk_g)   # edges per global block
    order = np.argsort(-bc, kind="stable")   # blocks by descending count
    # slot s gets ranked blocks [8s, 8s+8); core = position in the group
    block_of = np.empty((NCORES, nblk), dtype=np.int64)
    block_core = np.empty(nblk_g, dtype=np.int64)
    block_slot = np.empty(nblk_g, dtype=np.int64)
    for s in range(nblk):
        grp = order[s * NCORES:(s + 1) * NCORES]
        block_of[:, s] = grp
        block_core[grp] = np.arange(NCORES)
        block_slot[grp] = s

    core = block_core[gb]
    slot = block_slot[gb]

    gid = (core * nw_ + wine) * nblk + slot
    counts = np.bincount(gid, minlength=NCORES * nw_ * nblk)
    counts = counts.reshape(NCORES, nw_, nblk)
    G = -(-counts.max(axis=0) // P)          # ceil tiles per (w, slot)
    G[0] = np.maximum(G[0], 1)               # w=0 flush initializes acc

    # chunk assembly (identical across cores): slots ordered by expected
    # padding ascending; whole slots per chunk, <= MAX_CHUNK_TILES tiles.
    chunk_slots = []
    for w in range(nw_):
        pad = G[w] * P - counts[:, w, :].mean(axis=0)
        chunks, cur, curt = [], [], 0
        for s in np.argsort(pad, kind="stable"):
            g = int(G[w][s])
            if curt + g > cfg.MAX_CHUNK_TILES and cur:
                chunks.append(cur)
                cur, curt = [], 0
            cur.append(int(s))
            curt += g
        if cur:
            chunks.append(cur)
        chunk_slots.append(chunks)

    per_core = []
    for c in range(NCORES):
        m = core == c
        sl, db, we, bl = src_loc[m], dstb[m], wine[m], slot[m]
        key = we * nblk + bl
        o = np.argsort(key, kind="stable")
        ks = key[o]
        gcnt = np.bincount(ks, minlength=nw_ * nblk)
        gstart = np.concatenate([[0], np.cumsum(gcnt)[:-1]])
        within = np.arange(ks.size) - gstart[ks]
        maps = {}
        for w in range(nw_):
            slot_seq = [s for ch in chunk_slots[w] for s in ch]
            off = np.zeros(nblk, dtype=np.int64)
            pos_acc = 0
            for s in slot_seq:
                off[s] = pos_acc
                pos_acc += int(G[w][s]) * P
            ia = np.zeros(pos_acc, dtype=np.int16)          # pad -> row 0
            da = np.full(pos_acc, -1.0, dtype=np.float32)   # pad -> no match
            sel = we[o] == w
            pos = off[bl[o][sel]] + within[sel]
            ia[pos] = sl[o][sel]
            da[pos] = db[o][sel]
            # trailing pads of each chunk -> idx=-1 (ucode trims them).
            # DISABLED: the deployed HW ucode appears to lack the
            # negative-trim path (crashes); keep pads at idx=0.
            if False:
                t0 = 0
                for ch in chunk_slots[w]:
                    ntile = sum(int(G[w][s]) for s in ch)
                    a, b = t0 * P, (t0 + ntile) * P
                    real = np.nonzero(da[a:b] >= 0)[0]
                    last = int(real[-1]) + 1 if real.size else 0
                    ia[a + last:b] = -1
                    t0 += ntile
            maps[f"idx{w}"] = np.ascontiguousarray(
                np.tile(ia.reshape(-1, 16).T, (8, 1)))
            maps[f"dstb{w}"] = np.ascontiguousarray(da.reshape(-1, P).T)
        per_core.append(maps)
    return G, chunk_slots, per_core, block_of


def host_inputs(cfg: Cfg, input_feat, weight, cj, ci, block_of):
    """Shared (replicated) device inputs + per-core civ (slot layout)."""
    N, dout, nblk = cfg.N, cfg.D_OUT, cfg.nblk
    xt = np.ascontiguousarray(
        np.asarray(input_feat, dtype=np.float32).T.astype(BF16))
    wgt = np.ascontiguousarray(np.asarray(weight, dtype=np.float32).astype(BF16))
    # cj laid out per (window, tile): col w*tpw+tt, partition = row in tile
    cjf = np.asarray(cj, dtype=np.float32).reshape(-1)
    ncols = cfg.NWIN * cfg.tpw
    cjv = np.zeros((P, ncols), dtype=np.float32)
    for w in range(cfg.NWIN):
        wvals = cjf[w * cfg.win:(w + 1) * cfg.win]
        pad = np.zeros(cfg.tpw * P, dtype=np.float32)
        pad[:wvals.size] = wvals
        cjv[:, w * cfg.tpw:(w + 1) * cfg.tpw] = pad.reshape(cfg.tpw, P).T
    iot = np.ascontiguousarray(
        np.broadcast_to(np.arange(P, dtype=np.float32), (P, P)).astype(BF16))
    # ci in (core, slot) layout: civ[c][p, s] = ci[block_of[c,s]*128 + p]
    cip = np.zeros(NCORES * nblk * P, dtype=np.float32)
    cif = np.asarray(ci, dtype=np.float32).reshape(-1)
    cip[:N] = cif
    cip = cip.reshape(NCORES * nblk, P)
    civs = [np.ascontiguousarray(cip[block_of[c]].T) for c in range(NCORES)]
    return {"xt": xt, "wgt": wgt, "cjv": cjv, "iot": iot}, civs


# ---------------------------------------------------------------- device IR

def tile_blocks(cfg: Cfg, G, chunk_slots, w):
    """Per-tile (slot, k, g) in chunk-major emission order for window w."""
    out = []
    for ch in chunk_slots[w]:
        for b in ch:
            g = int(G[w][b])
            for k in range(g):
                out.append((b, k, g))
    return out


def build_nc(cfg: Cfg, G, chunk_slots):
    f32, bf16, i16 = mybir.dt.float32, mybir.dt.bfloat16, mybir.dt.int16
    dout, nblk, win, tpw = cfg.D_OUT, cfg.nblk, cfg.win, cfg.tpw
    tiles_w = [int(sum(G[w])) for w in range(cfg.NWIN)]

    nc = bacc.Bacc("TRN2", target_bir_lowering=False, debug=False,
                   num_swdge_queues=cfg.NQUEUES,
                   dynamic_dma_scratch_size=cfg.SCRATCH)
    xt = nc.dram_tensor("xt", [D_IN, cfg.N], bf16, kind="ExternalInput")
    wgt = nc.dram_tensor("wgt", [D_IN, dout], bf16, kind="ExternalInput")
    cjv = nc.dram_tensor("cjv", [P, cfg.NWIN * tpw], f32, kind="ExternalInput")
    civ = nc.dram_tensor("civ", [P, nblk], f32, kind="ExternalInput")
    iot = nc.dram_tensor("iot", [P, P], bf16, kind="ExternalInput")
    idx_t = [nc.dram_tensor(f"idx{w}", [P, tiles_w[w] * 8], i16,
                            kind="ExternalInput") for w in range(cfg.NWIN)]
    dstb_t = [nc.dram_tensor(f"dstb{w}", [P, tiles_w[w]], f32,
                             kind="ExternalInput") for w in range(cfg.NWIN)]
    hw_t = [nc.dram_tensor(f"h{w}", [tpw * P, HROW], bf16)
            for w in range(cfg.NWIN)]
    out_t = nc.dram_tensor("out", [nblk * P, dout], f32, kind="ExternalOutput")

    with tile.TileContext(nc) as tc:
        with (
            tc.tile_pool(name="const", bufs=1) as cpool,
            tc.tile_pool(name="xt", bufs=3) as xpool,
            tc.tile_pool(name="hs", bufs=3) as hpool,
            tc.tile_pool(name="idx", bufs=2) as ipool,
            tc.tile_pool(name="msg", bufs=2) as mpool,
            tc.tile_pool(name="oh", bufs=6) as opool,
            tc.tile_pool(name="ps", bufs=4, space="PSUM") as pspool,
            tc.tile_pool(name="acc", bufs=1) as apool,
        ):
            wgt_sb = cpool.tile([P, dout], bf16, tag="wgt")
            nc.sync.dma_start(out=wgt_sb[:], in_=wgt[:])
            iota_sb = cpool.tile([P, P], bf16, tag="iot")
            nc.sync.dma_start(out=iota_sb[:], in_=iot[:])
            cj_sb = cpool.tile([P, cfg.NWIN * tpw], f32, tag="cj")
            nc.sync.dma_start(out=cj_sb[:], in_=cjv[:])
            ci_sb = cpool.tile([P, nblk], f32, tag="ci")
            nc.sync.dma_start(out=ci_sb[:], in_=civ[:])
            acc = apool.tile([P, nblk * dout], f32, tag="acc")

            def emit_super(w, s):
                """Phase 1: one supertile of h_w = (X @ W) * cj (bf16 out)."""
                ncols = min(cfg.SUPER, win - s)
                nsub = math.ceil(ncols / P)
                xt_sb = xpool.tile([P, cfg.SUPER], bf16, tag="xt")
                nc.sync.dma_start(out=xt_sb[:, :ncols],
                                  in_=xt[:, w * win + s:w * win + s + ncols])
                hs_sb = hpool.tile([P, (cfg.SUPER // P) * dout], bf16, tag="hs")
                for q in range(nsub):
                    rows = min(P, ncols - q * P)
                    ps = pspool.tile([P, dout], f32, tag="ps1")
                    nc.tensor.matmul(
                        out=ps[:rows],
                        lhsT=xt_sb[:, q * P:q * P + rows],
                        rhs=wgt_sb[:],
                        start=True, stop=True)
                    tt = (s + q * P) // P  # tile index within window
                    nc.scalar.mul(
                        hs_sb[:rows, q * dout:(q + 1) * dout],
                        ps[:rows],
                        cj_sb[:rows, w * tpw + tt:w * tpw + tt + 1])
                nfull = ncols // P  # full 128-row subtiles
                if nfull:
                    nc.sync.dma_start(
                        out=hw_t[w][s:s + nfull * P, :dout].rearrange(
                            "(q p) f -> p q f", p=P),
                        in_=hs_sb[:, :nfull * dout].rearrange(
                            "p (q f) -> p q f", f=dout))
                if nsub > nfull:  # ragged tail: only the valid rows
                    rows = ncols - nfull * P
                    nc.sync.dma_start(
                        out=hw_t[w][s + nfull * P:s + ncols, :dout],
                        in_=hs_sb[:rows, nfull * dout:nsub * dout])

            st = {"ps": None}

            def emit_chunk(w, t0, nt, tb, idx_sb, dst_sb, qn):
                """Phase 2: gather one chunk of edges and matmul-scatter it."""
                ne = nt * P
                msg = mpool.tile([P, nt * HROW], bf16, tag="msg")
                nc.gpsimd.dma_gather(
                    msg[:].rearrange("p (t f) -> p t f", f=HROW),
                    hw_t[w][:],
                    idx_sb[:, t0 * 8:(t0 + nt) * 8],
                    ne, ne, HROW,
                    queue_num=qn)
                for t in range(nt):
                    b, k, g = tb[t0 + t]
                    if k == 0:
                        st["ps"] = pspool.tile([P, dout], f32, tag="ps2",
                                               name="ps2")
                    ps = st["ps"]
                    oh = opool.tile([P, P], bf16, tag="oh")
                    nc.vector.tensor_tensor(
                        out=oh[:],
                        in0=dst_sb[:, t0 + t:t0 + t + 1].to_broadcast([P, P]),
                        in1=iota_sb[:],
                        op=mybir.AluOpType.is_equal)
                    nc.tensor.matmul(
                        out=ps[:],
                        lhsT=oh[:],
                        rhs=msg[:, t * HROW:t * HROW + dout],
                        start=(k == 0), stop=(k == g - 1))
                    if k == g - 1:
                        if w == 0:
                            nc.vector.tensor_copy(
                                out=acc[:, b * dout:(b + 1) * dout], in_=ps[:])
                        else:
                            nc.vector.tensor_add(
                                out=acc[:, b * dout:(b + 1) * dout],
                                in0=acc[:, b * dout:(b + 1) * dout],
                                in1=ps[:])

            super_starts = list(range(0, win, cfg.SUPER))
            for s in super_starts:
                emit_super(0, s)
            qn = 0
            for w in range(cfg.NWIN):
                idx_sb = ipool.tile([P, tiles_w[w] * 8], i16, tag="idx")
                nc.sync.dma_start(out=idx_sb[:], in_=idx_t[w][:])
                dst_sb = ipool.tile([P, tiles_w[w]], f32, tag="dstb")
                nc.sync.dma_start(out=dst_sb[:], in_=dstb_t[w][:])

                tb = tile_blocks(cfg, G, chunk_slots, w)
                sizes = [sum(int(G[w][s]) for s in ch) for ch in chunk_slots[w]]
                starts = [0]
                for sz in sizes[:-1]:
                    starts.append(starts[-1] + sz)
                # software-pipeline: weave window w+1's phase 1 between
                # window w's gather/scatter chunks so PE/ACT/DMA stay busy
                # while the gather queue drains
                nxt = super_starts if w + 1 < cfg.NWIN else []
                frac, si = 0.0, 0
                ratio = len(nxt) / max(1, len(starts))
                for ci_, t0 in enumerate(starts):
                    emit_chunk(w, t0, sizes[ci_], tb, idx_sb, dst_sb, qn)
                    qn = (qn + 1) % cfg.NQUEUES
                    frac += ratio
                    while frac >= 1.0 and si < len(nxt):
                        emit_super(w + 1, nxt[si])
                        si += 1
                        frac -= 1.0
                while si < len(nxt):
                    emit_super(w + 1, nxt[si])
                    si += 1

            # ---- epilogue: scale by ci, store ----
            for b in range(nblk):
                nc.vector.tensor_mul(
                    out=acc[:, b * dout:(b + 1) * dout],
                    in0=acc[:, b * dout:(b + 1) * dout],
                    in1=ci_sb[:, b:b + 1].to_broadcast([P, dout]))
            nc.sync.dma_start(
                out=out_t[:].rearrange("(b p) f -> p b f", p=P),
                in_=acc[:].rearrange("p (b f) -> p b f", f=dout))
    nc.compile()
    return nc


# ---------------------------------------------------------------- entry

def run(cfg: Cfg, input_feat, weight, cj, ci, src_idx, dst_idx, **run_kwargs):
    G, chunk_slots, per_core, block_of = shard_edges(cfg, src_idx, dst_idx)
    shared, civs = host_inputs(cfg, input_feat, weight, cj, ci, block_of)
    nc = build_nc(cfg, G, chunk_slots)
    in_maps = []
    for c in range(NCORES):
        m = dict(shared)
        m["civ"] = civs[c]
        m.update(per_core[c])
        in_maps.append(m)
    res = run_bass_kernel_spmd(nc, in_maps, core_ids=list(range(NCORES)),
                               **run_kwargs)
    # un-permute: core c slot s holds global dst block block_of[c, s]
    full = np.zeros((NCORES * cfg.nblk * P, cfg.D_OUT), dtype=np.float32)
    blk_rows = full.reshape(NCORES * cfg.nblk, P, cfg.D_OUT)
    for c in range(NCORES):
        o = res.results[c]["out"].reshape(cfg.nblk, P, cfg.D_OUT)
        blk_rows[block_of[c]] = o
    return full[:cfg.N], res


def kernel(input_feat, weight, cj, ci, src_idx, dst_idx):
    out, _ = run(CFG, input_feat, weight, cj, ci, src_idx, dst_idx)
    return out
